# revision 3
# baseline (speedup 1.0000x reference)
"""LMHSA (downsampled-KV MHSA + DLA attention refinement) on 8 Trainium2 cores.

Takes FULL unsharded inputs, returns FULL output. Data-parallel over the
batch dim: B=16 -> 8 NeuronCores x 2 batches each, via one pmap-compiled
XLA/Neuron program. Conv-free formulation (reshape+einsum downsample, 1x1
convs as matmuls, 3x3 depthwise as 9 shifted adds) so neuronx-cc handles it.

Falls back to jit on CPU if no axon/neuron devices are reachable.
"""

import os

os.environ.setdefault('NEURON_COMPILE_CACHE_URL', '/root/.cache/neuron_compile_cache')

import numpy as np
import ml_dtypes
import jax, jax.numpy as jnp

try:
    jax.config.update('jax_compilation_cache_dir', '/root/.cache/jaxcache')
    jax.config.update('jax_persistent_cache_min_compile_time_secs', 0.0)
    jax.config.update('jax_persistent_cache_min_entry_size_bytes', 0)
except Exception:
    pass

B, C, H, W = 16, 512, 56, 56
K = 8
HEADS = 8
EXP = 3
HID = 24
HD = 64
SCALE = HD ** -0.5
N = H * W
HK, WK = H // K, W // K
NK = HK * WK
EPS = 1e-5


def _gn(x, scale, bias, groups):
    b, ch, n, m = x.shape
    xg = x.reshape(b, groups, ch // groups, n, m)
    mu = xg.mean(axis=(2, 3, 4), keepdims=True)
    var = (xg * xg).mean(axis=(2, 3, 4), keepdims=True) - mu * mu
    xg = (xg - mu) * jax.lax.rsqrt(var + EPS)
    x = xg.reshape(b, ch, n, m)
    return x * scale[None, :, None, None] + bias[None, :, None, None]


def _swish(x):
    return x * jax.nn.sigmoid(x)


def _fwd(x, q_w, down_k, kv_w, proj_w, proj_b, rel_bias,
         ew, gn1_s, gn1_b, dw, gn2_s, gn2_b, rw, gn3_s, gn3_b):
    b = x.shape[0]
    x = x.astype(jnp.float32)
    xr = x.reshape(b, C, HK, K, WK, K)
    kvx = jnp.einsum('bchkwl,ckl->bchw', xr, down_k).reshape(b, C, NK)
    kv = kvx.transpose(0, 2, 1) @ kv_w                    # (b,NK,2C)
    kv = kv.reshape(b, NK, 2, HEADS, HD).transpose(2, 0, 3, 1, 4)
    k, v = kv[0], kv[1]                                   # (b,8,49,64)

    q = x.reshape(b, C, N).transpose(0, 2, 1) @ q_w       # (b,N,C)
    q = q.reshape(b, N, HEADS, HD).transpose(0, 2, 1, 3)  # (b,8,N,64)

    attn = jnp.einsum('bhnd,bhmd->bhnm', q, k) * SCALE + rel_bias[None, None]
    attn = jax.nn.softmax(attn, axis=-1)                  # (b,8,N,49)

    a2 = attn.reshape(b, HEADS, N * NK)
    y1 = jnp.einsum('eh,bhs->bes', ew, a2).reshape(b, HID, N, NK)
    z1 = _swish(_gn(y1, gn1_s, gn1_b, EXP))

    zp = jnp.pad(z1, ((0, 0), (0, 0), (1, 1), (1, 1)))
    y2 = jnp.zeros_like(z1)
    for di in range(3):
        for dj in range(3):
            y2 = y2 + zp[:, :, di:di + N, dj:dj + NK] * dw[None, :, di, dj, None, None]
    z2 = _swish(_gn(y2, gn2_s, gn2_b, EXP))

    y3 = jnp.einsum('he,bes->bhs', rw, z2.reshape(b, HID, N * NK)).reshape(b, HEADS, N, NK)
    a_dla = _gn(y3, gn3_s, gn3_b, 1)

    out = jnp.einsum('bhnm,bhmd->bhnd', a_dla, v)
    out = out.transpose(0, 2, 1, 3).reshape(b, N, C)
    out = out @ proj_w + proj_b
    return out.reshape(b, C, H, W).astype(jnp.bfloat16)


_CACHE = {}


def _get_pmap():
    if 'pf' in _CACHE:
        return _CACHE['pf'], _CACHE['nd']
    devs = jax.devices()
    nd = min(8, len(devs))
    if B % nd != 0:
        nd = 1
    pf = jax.pmap(
        _fwd,
        in_axes=(0,) + (None,) * 15,
        devices=devs[:nd],
    )
    _CACHE['pf'] = pf
    _CACHE['nd'] = nd
    return pf, nd


def kernel(x, q_w, down_w, kv_w, proj_w, proj_b, rel_bias,
           expand_w, gn1_s, gn1_b, dw_w, gn2_s, gn2_b,
           reduce_w, gn3_s, gn3_b):
    x = np.ascontiguousarray(np.asarray(x, np.float32)).astype(ml_dtypes.bfloat16)
    w = [
        np.asarray(q_w, np.float32),
        np.ascontiguousarray(np.asarray(down_w, np.float32)[:, 0]),
        np.asarray(kv_w, np.float32),
        np.asarray(proj_w, np.float32),
        np.asarray(proj_b, np.float32),
        np.asarray(rel_bias, np.float32),
        np.ascontiguousarray(np.asarray(expand_w, np.float32)[:, :, 0, 0]),
        np.asarray(gn1_s, np.float32), np.asarray(gn1_b, np.float32),
        np.ascontiguousarray(np.asarray(dw_w, np.float32)[:, 0]),
        np.asarray(gn2_s, np.float32), np.asarray(gn2_b, np.float32),
        np.ascontiguousarray(np.asarray(reduce_w, np.float32)[:, :, 0, 0]),
        np.asarray(gn3_s, np.float32), np.asarray(gn3_b, np.float32),
    ]
    try:
        pf, nd = _get_pmap()
        xs = x.reshape(nd, B // nd, C, H, W)
        out = pf(xs, *w)
        res = np.asarray(out).reshape(B, C, H, W)
    except Exception:
        if 'jf' not in _CACHE:
            _CACHE['jf'] = jax.jit(_fwd, backend='cpu')
        res = np.asarray(_CACHE['jf'](x, *w))
    return np.ascontiguousarray(res.astype(np.float32))
